# revision 65
# baseline (speedup 1.0000x reference)
"""Trainium2 Bass kernel for batched cross-attention + multiscale sigmoid gate.

Reference computation (per batch b):
    q = x1 @ Wq.T + bq ; k = x2 @ Wk.T + bk ; v = x2 @ Wv.T + bv
    attn = softmax(q @ k.T, axis=-1)              (unscaled)
    out = attn @ v
    s = out @ (W1+W2+W3).T + (b1+b2+b3)
    out = out * sigmoid(s)
    return gamma * out + x1

Key algebraic fold: softmax is invariant to per-row constants, so
    softmax(q k^T) = softmax(x1 (Wq^T Wk) x2^T + 1 (bq^T Wk) x2^T)
(the x1 Wq^T bk and bq.bk terms are constant along the softmax axis and
cancel). The kernel computes q' = x1 @ M + c with M = Wq^T Wk and
c = bq @ Wk folded on the host, and uses the already-resident fp8 x2
directly as the key matrix: the entire K projection (128 of 1024 matmul
instructions per batch) and its drains vanish. bv is folded into the V
drain as a broadcast-tensor add, so the PV drains are bias-free wide ops.

Strategy: pure data-parallel over batch (16 batches -> 8 cores x 2), no
collectives. Everything on-chip is kept transposed ([feature, token]) so
all matmuls contract over the partition dim with zero on-device
transposes. ALL matmuls run fp8e4m3 DoubleRow (2x contraction tiles per
instruction): Q' projection, QK^T energy (keys = raw x2), PV, and the
gate. Weights are pre-scaled x32 on the host so fp8 encodings stay out
of the subnormal range; every descale folds into an existing epilogue.

Pipeline: two-deep deferral -- in each attention window (energy of block
ib) the PV of block ib-1 and the gate of block ib-2 are flushed, so PV
drains get a full extra window before the gate contracts over them and
nothing on the slow queues is latency-critical. At a batch boundary the
previous batch's last PV+gate run inside the first attention window
(v_sb is double-buffered for the WAR hazard); the first batch instead
defers its pb3 V-projection groups there.

Engine assignment (GPSIMD/Pool cannot touch PSUM and cannot run
tensor_scalar/STT on TRN2 -- hardware-verified constraints):
  ScalarE/ACT: exps (high-priority: latency-critical), Q' drains, all
    four wide PV drains, tanh (sigmoid via (tanh+1)/2, same ACT table as
    Exp so no table reloads).
  VectorE/DVE: den accumulation (1024-wide bf16 2x adds), reciprocal,
    softmax normalize (5 of 8 j-pair TTs), V drains (TT + broadcast bv),
    wide fin STTs.
  GpSimd/Pool: partition all-reduce of the denominator and the last 3
    normalize pairs.
Output is written fp8 (the gated path is ~3% of the result); the
gamma/2 factor and the residual add (+x1) are applied on the host.

Numerics (hardware-validated): full-output rel err 3.7e-3 vs f32
reference (budget 2e-2) -- slightly better than a separate-Q/K baseline
since only one fp8 weight quantization enters the energy path.
TimelineSim per-core estimate: 226.4us (baseline 290.8us).
"""

import math

import numpy as np
import ml_dtypes

import concourse.tile as tile
from concourse import mybir, bacc
from concourse.bass_isa import ReduceOp

P = 128
F32 = mybir.dt.float32
BF16 = mybir.dt.bfloat16
F8 = mybir.dt.float8e4
AF = mybir.ActivationFunctionType
OP = mybir.AluOpType
DR = mybir.MatmulPerfMode.DoubleRow

# full problem shape (hardcoded per harness contract)
B_FULL, N_FULL, D_FULL = 16, 2048, 1024
MARKS = None  # debug: list to receive (label, next_instr_id) emission marks
N_CORES = 8
SHIFT = 64.0
WSC = 32.0            # host-side weight scale (fp8 subnormal dodge)
WSC_M = 32.0          # scale for M = Wq^T Wk
WSC_Q = 32.0          # scale of q' as stored in fp8


def build(BPC, N, D, gamma, shift=SHIFT, reps=1):
    """Build the per-core Bass graph. BPC = batches per core."""
    DC = D // P          # feature chunks of 128
    NJ = N // P          # key tiles of 128
    PB = min(512, N)     # projection n-block
    NPB = N // PB
    IB = min(512, N)     # attention i-block (query block)
    NIB = N // IB
    KH = math.ceil(D / 512)  # V-projection k halves
    assert DC % 2 == 0 and NJ % 2 == 0

    nc = bacc.Bacc("TRN2", target_bir_lowering=False, debug=False,
                   num_devices=N_CORES)

    x1t_d = nc.declare_dram_parameter("x1t_8", [BPC, D, N], F8, isOutput=False)
    x2t_d = nc.declare_dram_parameter("x2t_8", [BPC, D, N], F8, isOutput=False)
    m_d = nc.declare_dram_parameter("m_t", [D, D], F8, isOutput=False)
    wv_d = nc.declare_dram_parameter("wv_t", [D, D], F8, isOutput=False)
    ws_d = nc.declare_dram_parameter("ws_t", [D, D], F8, isOutput=False)
    # bias slots: 0 = WSC_Q * (bq @ Wk), 1 = unused, 2 = (b1+b2+b3)/2
    ball_d = nc.declare_dram_parameter("b_all", [P, 3, DC], F32, isOutput=False)
    # WSC*bv broadcast across partitions, [P, KH, 512] (feature = kh*512+m):
    # added at the V drain so the PV drains are bias-free wide ops
    bvb_d = nc.declare_dram_parameter("bv_bc", [P, KH, 512], BF16,
                                      isOutput=False)
    out_ext = nc.declare_dram_parameter("out", [BPC, D, N], F8, isOutput=True)

    def r3(ap):  # [D, N] dram view -> [p, dc, n]
        return ap.rearrange("(c p) n -> p c n", p=P)

    def mark(label):
        if MARKS is not None:
            MARKS.append((label, nc.next_id()))

    def mm8(pst, lhsT3, rhs3, start, stop):
        """fp8 DoubleRow matmul over 2 contraction chunk-tiles."""
        nc.tensor.matmul(pst, lhsT=lhsT3, rhs=rhs3, start=start, stop=stop,
                         perf_mode=DR)

    with tile.TileContext(nc) as tc:
        with (
            tc.tile_pool(name="w8", bufs=1) as w8,
            tc.tile_pool(name="consts", bufs=1) as consts,
            tc.tile_pool(name="xin", bufs=5) as xin,
            tc.tile_pool(name="x2r", bufs=2) as x2r,
            tc.tile_pool(name="kv", bufs=1) as kvpool,
            tc.tile_pool(name="vp", bufs=2) as vpool,
            tc.tile_pool(name="pall", bufs=2) as pall_pool,
            tc.tile_pool(name="p8", bufs=2) as p8_pool,
            tc.tile_pool(name="obf", bufs=2) as obf_pool,
            tc.tile_pool(name="small", bufs=2) as small,
            tc.tile_pool(name="gp", bufs=4) as gpool,
            tc.tile_pool(name="fin", bufs=3) as finpool,
            # all PSUM flows through 2-bank [P, 2, IB] tiles: 4 bufs span
            # the whole 16KB, doubling the rotation window vs 1-bank tiles
            tc.tile_pool(name="ps2", bufs=4, space="PSUM") as ps2p,
        ):
            # constants / biases
            negshift = consts.tile([P, 1], F32)
            nc.vector.memset(negshift[:], -shift)

            # weights: fp8, resident for the whole kernel (batch-invariant).
            # Startup: column-sliced DMAs ordered so the first Q' matmul
            # group's operands (M columns 0:256 + x1 chunk half) land first;
            # wv/ws stream in behind the first projection groups.
            m_sb = w8.tile([P, DC, D], F8, tag="m")
            wv_sb = w8.tile([P, DC, D], F8, tag="wv")
            ws_sb = w8.tile([P, DC, D], F8, tag="ws")
            H = DC // 2
            nsl0 = slice(0, PB)
            x1t0 = xin.tile([P, DC, PB], F8, tag="xin")
            x2_sb0 = x2r.tile([P, DC, N], F8, tag="x2")
            # startup critical path: the first Q' mm8 needs only x1 chunks
            # 0:2 + M columns 0:128. Spread the first DMAs across the
            # SP/ACT/DVE dispatch queues (each dispatch is ~0.6us serial per
            # queue) and order by first use.
            m3 = r3(m_d.ap())
            x13 = r3(x1t_d[0])
            b_all = consts.tile([P, 3, DC], F32)
            # HWDGE descriptor generation is the serial resource here
            # (~0.6us per DMA across both queues): few, large transfers,
            # ordered by first use. b_all is tiny and gates the first Q
            # drain, so it leads the ACT queue.
            nc.sync.dma_start(x1t0[:, 0:H], x13[:, 0:H, nsl0])
            # first M column block via SWDGE (gpsimd): its generation runs
            # on the idle Pool engine, off the serial HWDGE path, so the
            # very first matmul's weights land ~1.5us earlier
            nc.gpsimd.dma_start(m_sb[:, :, 0:P], m3[:, :, 0:P])
            nc.scalar.dma_start(m_sb[:, :, P:2 * P], m3[:, :, P:2 * P])
            nc.scalar.dma_start(b_all[:], ball_d[:])
            nc.sync.dma_start(x1t0[:, H:], x13[:, H:, nsl0])
            nc.scalar.dma_start(m_sb[:, :, 2 * P:4 * P], m3[:, :, 2 * P:4 * P])
            nc.sync.dma_start(m_sb[:, :, 4 * P:D], m3[:, :, 4 * P:D])
            bv_bc = consts.tile([P, KH, 512], BF16)
            nc.scalar.dma_start(bv_bc[:], bvb_d[:])
            nc.sync.dma_start(x2_sb0[:, :, nsl0], r3(x2t_d[0])[:, :, nsl0])

            def emit_pv(p_lo, v_cur, last=False):
                # out = (P @ V')/32 + bv; kc pairs share a 2-bank PSUM
                # tile. Drains go to GpSimd: with the gate deferred a full
                # window, nothing on the Pool queue is latency-critical.
                # For the tail flush (last=True) the drains rotate over
                # ACT/DVE/Pool since every engine is idle by then.
                out_lo = obf_pool.tile([P, DC, IB], F8, tag="obf")
                mark("flushPV")
                # kc2 descending: the first groups' drains go to ACT (idle
                # right after the exps), the last to DVE; the gate also
                # contracts descending, so every chunk pair is drained just
                # ahead of its first use
                for kc2 in range(DC // 2 - 1, -1, -1):
                    o_ps = ps2p.tile([P, 2, IB], F32, tag="ps2")
                    for h in (0, 1):
                        kc = 2 * kc2 + h
                        kh, ko = divmod(kc * P, 512)
                        for jp in range(NJ // 2):
                            mm8(o_ps[:, h],
                                v_cur[:, 2 * jp:2 * jp + 2, kh, ko:ko + P],
                                p_lo[:, 2 * jp:2 * jp + 2, :],
                                start=(jp == 0), stop=(jp == NJ // 2 - 1))
                    # bias-free wide drain (bv rides in v_sb): the last
                    # chunk pair goes to ACT so its PSUM tile -- the next
                    # gate group's buffer -- frees promptly; GPSIMD cannot
                    # read PSUM on TRN2.
                    osl = out_lo[:, 2 * kc2:2 * kc2 + 2, :]
                    if not last:
                        nc.scalar.activation(osl, o_ps[:], AF.Identity,
                                             scale=1.0 / WSC)
                    else:
                        # (all drains DVE in the tail: ACT must be free to
                        # start the final tanh chain immediately)
                        nc.vector.tensor_scalar(osl, o_ps[:], 1.0 / WSC,
                                                None, OP.mult)
                return out_lo

            def gate_final(out_lo, b_o, ib, last=False):
                isl = slice(ib * IB, (ib + 1) * IB)
                o3 = r3(out_ext[b_o])
                mark("flushGate")
                for ec2 in range(DC // 2):
                    g_ps = ps2p.tile([P, 2, IB], F32, tag="ps2")
                    for h in (0, 1):
                        ec = 2 * ec2 + h
                        for dc2 in range(DC // 2 - 1, -1, -1):
                            mm8(g_ps[:, h],
                                ws_sb[:, 2 * dc2:2 * dc2 + 2, ec * P:(ec + 1) * P],
                                out_lo[:, 2 * dc2:2 * dc2 + 2, :],
                                start=(dc2 == DC // 2 - 1), stop=(dc2 == 0))
                    fin = finpool.tile([P, 2, IB], F8, tag="fin")
                    g_sb = gpool.tile([P, 2, IB], BF16, tag="g")
                    for h in (0, 1):
                        ec = 2 * ec2 + h
                        # sigmoid(x) = 0.5*tanh(x/2) + 0.5 ; Tanh shares the
                        # ACT table with Exp/Identity, so no table reloads.
                        # bias slot 2 is bs/2; fin = (tanh+1)*out and the
                        # remaining gamma/2 factor applies on the host.
                        nc.scalar.activation(g_sb[:, h], g_ps[:, h], AF.Tanh,
                                             bias=b_all[:, 2, ec:ec + 1],
                                             scale=1.0 / (2.0 * WSC))
                        if last:
                            # tail: per-half fin + DMA pipeline with the
                            # tanh chain instead of waiting for the pair
                            nc.vector.scalar_tensor_tensor(
                                fin[:, h], g_sb[:, h], 1.0,
                                out_lo[:, ec, :], OP.add, OP.mult)
                            nc.sync.dma_start(o3[:, ec, isl], fin[:, h])
                    if not last:
                        # one wide fin per pair on DVE (GPSIMD cannot STT);
                        # deprioritized: it only feeds the output DMA
                        with tc.high_priority(offset=-300):
                            nc.vector.scalar_tensor_tensor(
                                fin[:], g_sb[:], 1.0,
                                out_lo[:, 2 * ec2:2 * ec2 + 2, :],
                                OP.add, OP.mult)
                        # one 2-chunk DMA per pair
                        nc.sync.dma_start(o3[:, 2 * ec2:2 * ec2 + 2, isl],
                                          fin[:])

            # Two-deep software pipeline: at any flush point the pending
            # PV (one window behind) is emitted, then ONE queued gate
            # (two windows behind, FIFO). PV drains thus get a whole
            # extra window before the gate contracts over them. At a
            # batch boundary the PV flush and its gate split across two
            # windows so the first attention window keeps PE fed.
            import collections as _c
            first = True
            pv_pending = None    # (p_lo, v_sb, b, ib)
            gates = _c.deque()   # queued (out_lo, b, ib)

            def flush_point(last=False, gates_too=True):
                nonlocal pv_pending
                if pv_pending is not None:
                    p_lo, v_cur, b_o, ib_o = pv_pending
                    gates.append((emit_pv(p_lo, v_cur, last=last), b_o, ib_o))
                    pv_pending = None
                if gates_too and gates:
                    o_lo, b_o, ib_o = gates.popleft()
                    gate_final(o_lo, b_o, ib_o, last=last)

            pre_x = None     # next batch's first x1 tile, prefetched
            blist = [bb for _ in range(reps) for bb in range(BPC)]
            for bi, b in enumerate(blist):
                last_batch = bi == len(blist) - 1
                # ---- phase 1: Q' + V projections (all fp8 DoubleRow) ----
                qt_sb = kvpool.tile([P, DC, N], F8, tag="qt")
                if first:
                    x2_sb = x2_sb0
                else:
                    x2_sb = pre_x2
                v_sb = None  # allocated in pb0 (pool is double-buffered)
                x1_tiles = {}
                defer_v3 = bi == 0  # first batch: pb3 V runs in window ib0

                def prefetch_rest():
                    # dispatch the whole batch's remaining x tiles up
                    # front, alternating HWDGE queues: transfers overlap
                    # the full 27us projection phase
                    # all on the SP queue: scalar.dma_start dispatches
                    # occupy the ACT sequencer (~0.7us each) and delay the
                    # Q drains behind them
                    for pp in range(1, NPB):
                        ns = slice(pp * PB, (pp + 1) * PB)
                        x1p = xin.tile([P, DC, PB], F8, tag="xin")
                        nc.sync.dma_start(x1p[:], r3(x1t_d[b])[:, :, ns])
                        nc.sync.dma_start(x2_sb[:, :, ns],
                                          r3(x2t_d[b])[:, :, ns])
                        x1_tiles[pp] = x1p

                for pb in range(NPB):
                    nsl = slice(pb * PB, (pb + 1) * PB)
                    mark(f"b{bi}.pb{pb}")
                    if pb == 0:
                        x1t = x1t0 if first else pre_x
                        pre_x = None
                    else:
                        x1t = x1_tiles.pop(pb)

                    def q_pair(ec2):
                        # two Q' chunks in one 2-bank tile, ACT drains
                        pst = ps2p.tile([P, 2, PB], F32, tag="ps2")
                        for h in (0, 1):
                            e = 2 * ec2 + h
                            for dc2 in range(DC // 2):
                                mm8(pst[:, h],
                                    m_sb[:, 2 * dc2:2 * dc2 + 2, e * P:(e + 1) * P],
                                    x1t[:, 2 * dc2:2 * dc2 + 2, :],
                                    start=(dc2 == 0), stop=(dc2 == DC // 2 - 1))
                        for h in (0, 1):
                            e = 2 * ec2 + h
                            nc.scalar.activation(qt_sb[:, e, nsl],
                                                 pst[:, h], AF.Identity,
                                                 bias=b_all[:, 0, e:e + 1],
                                                 scale=WSC_Q / WSC_M)

                    def v_pair(js, vpb=None, pool_drain=False):
                        # V chunk [j, :] for one j-tile: both 512-wide k
                        # halves in one 2-bank tile, drained as a single
                        # 1024-wide copy (no bias; bv added at the PV
                        # drain). Deferred pairs drain on Pool to keep the
                        # attention-window DVE (softmax chain) clean.
                        if vpb is None:
                            vpb = pb
                        jt = vpb * (PB // P) + js
                        tok = slice(vpb * PB + js * P, vpb * PB + (js + 1) * P)
                        pst = ps2p.tile([P, 2, PB], F32, tag="ps2")
                        for kh in range(KH):
                            k0 = kh * 512
                            for dc2 in range(DC // 2):
                                mm8(pst[:, kh],
                                    x2_sb[:, 2 * dc2:2 * dc2 + 2, tok],
                                    wv_sb[:, 2 * dc2:2 * dc2 + 2, k0:k0 + 512],
                                    start=(dc2 == 0), stop=(dc2 == DC // 2 - 1))
                        nc.vector.tensor_tensor(v_sb[:, jt], pst[:],
                                                 bv_bc[:], OP.add)

                    if first and pb == 0:
                        # staged order: Q' groups first (only M + x1 gate
                        # the start), weight DMAs between groups
                        q_pair(0)
                        wv3 = r3(wv_d.ap())
                        nc.sync.dma_start(wv_sb[:, :, 0:512],
                                          wv3[:, :, 0:512])
                        nc.sync.dma_start(wv_sb[:, :, 512:D],
                                          wv3[:, :, 512:D])
                        prefetch_rest()
                        q_pair(1)
                        q_pair(2)
                        q_pair(3)
                        v_sb = vpool.tile([P, NJ, KH, 512], F8, tag="v")
                        for js in range(PB // P):
                            v_pair(js)
                        nc.sync.dma_start(ws_sb[:], r3(ws_d.ap()))
                        first = False
                    elif pb == 0:
                        # no flush here: the previous batch's PV(ib3) and
                        # gate(ib2) run inside window ib0 (v_sb is
                        # double-buffered so the new batch's V drains
                        # don't wait on PV(ib3) reading the old buffer)
                        v_sb = vpool.tile([P, NJ, KH, 512], F8, tag="v")
                        q_pair(0)
                        prefetch_rest()
                        for ec2 in range(1, DC // 2):
                            q_pair(ec2)
                            v_pair(ec2 - 1)
                        v_pair(DC // 2 - 1)
                    elif defer_v3 and pb == NPB - 1:
                        # first batch has no flush work in its first
                        # attention window; its PE would run ahead of the
                        # softmax chain and throttle on norm production.
                        # Keep pb3's V groups back and emit them there.
                        for ec2 in range(DC // 2):
                            q_pair(ec2)
                        pend_v = v_pair
                    else:
                        # interleave Q' pairs with V pairs so drains
                        # rotate over ACT and DVE
                        for ec2 in range(DC // 2):
                            q_pair(ec2)
                            v_pair(ec2)

                # ---- phase 2: attention + gate, per i-block ----
                for ib in range(NIB):
                    isl = slice(ib * IB, (ib + 1) * IB)
                    if ib == NIB - 1 and not last_batch:
                        # prefetch the next batch's first x1 tile and x2
                        # slice so its projections start without a DMA wait
                        nb = blist[bi + 1]
                        x1p = xin.tile([P, DC, PB], F8, tag="xin")
                        nc.sync.dma_start(x1p[:], r3(x1t_d[nb])[:, :, 0:PB])
                        pre_x2 = x2r.tile([P, DC, N], F8, tag="x2")
                        nc.sync.dma_start(pre_x2[:, :, 0:PB],
                                          r3(x2t_d[nb])[:, :, 0:PB])
                        pre_x = x1p
                    mark(f"b{bi}.ib{ib}.energy")
                    p_all = pall_pool.tile([P, NJ, IB], BF16, tag="pall")
                    den2 = small.tile([P, 2, IB], BF16, tag="den2")

                    # pass A: energy (fp8 DR, keys = raw x2) into 2-bank
                    # PSUM tiles so a single exp covers two j-tiles;
                    # denominator accumulated as 1024-wide bf16 2x-mode
                    # adds on DVE into den2, folded once at the end
                    for jh in range(NJ // 2):
                        ps2 = ps2p.tile([P, 2, IB], F32, tag="ps2")
                        for h in (0, 1):
                            j = 2 * jh + h
                            for dc2 in range(DC // 2):
                                mm8(ps2[:, h],
                                    x2_sb[:, 2 * dc2:2 * dc2 + 2,
                                          j * P:(j + 1) * P],
                                    qt_sb[:, 2 * dc2:2 * dc2 + 2, isl],
                                    start=(dc2 == 0),
                                    stop=(dc2 == DC // 2 - 1))
                        with tc.high_priority(offset=400):
                            nc.scalar.activation(
                                p_all[:, 2 * jh:2 * jh + 2, :],
                                ps2[:], AF.Exp,
                                bias=negshift[:, 0:1],
                                scale=1.0 / WSC_Q)
                        if jh == 0:
                            nc.vector.tensor_copy(den2[:],
                                                  p_all[:, 0:2, :])
                        else:
                            nc.vector.tensor_tensor(
                                den2[:], den2[:],
                                p_all[:, 2 * jh:2 * jh + 2, :], OP.add)

                    den_a = small.tile([P, IB], BF16, tag="dena")
                    nc.vector.tensor_tensor(den_a[:], den2[:, 0], den2[:, 1],
                                            OP.add)
                    # partition all-reduce on GpSimd (sum + broadcast in one
                    # op), then reciprocal on DVE.
                    den_all = small.tile([P, IB], F32, tag="denbf")
                    nc.gpsimd.partition_all_reduce(den_all[:], den_a[:], P,
                                                   ReduceOp.add)
                    rec_bc = small.tile([P, IB], F32, tag="recbc")
                    nc.vector.reciprocal(rec_bc[:], den_all[:])

                    # normalize P and cast to fp8, entirely on DVE and
                    # emitted BEFORE the flush: it only depends on recip
                    # (which lands early) and DVE carries no flush ops, so
                    # p_lo completes a full window before PV consumes it.
                    # normalize P pairs: the Pool share (tail pairs) goes
                    # ahead of the flush (it waits only on recip; fins have
                    # slack), the DVE share after it (so the flush's PV
                    # drains run early and free their PSUM tiles for the
                    # next window's rotation).
                    mark(f"b{bi}.ib{ib}.norm")
                    p_lo = p8_pool.tile([P, NJ, IB], F8, tag="p8")
                    rec2 = rec_bc[:].unsqueeze(1).broadcast_to((P, 2, IB))
                    n_gp = 3 if NJ >= 16 else 0
                    for jh in range(NJ // 2 - n_gp, NJ // 2):
                        nc.gpsimd.tensor_tensor(
                            p_lo[:, 2 * jh:2 * jh + 2, :],
                            p_all[:, 2 * jh:2 * jh + 2, :],
                            rec2, OP.mult)
                    flush_point()
                    with tc.high_priority(offset=400):
                        for jh in range(NJ // 2 - n_gp):
                            nc.vector.tensor_tensor(
                                p_lo[:, 2 * jh:2 * jh + 2, :],
                                p_all[:, 2 * jh:2 * jh + 2, :],
                                rec2, OP.mult)
                    if ib == 0 and defer_v3:
                        for js in range(PB // P):
                            pend_v(js, vpb=NPB - 1)
                        defer_v3 = False
                    if last_batch and ib == NIB - 1 and gates:
                        # pull the ib2 gate into this window: its tanh
                        # chain then overlaps the tail's PV matmuls
                        o_lo, b_o, ib_o = gates.popleft()
                        gate_final(o_lo, b_o, ib_o)
                    pv_pending = (p_lo, v_sb, b, ib)

            # tail: the pending gate first (its inputs are long ready, so
            # its tanh chain overlaps PV(ib3)'s matmuls), then PV of the
            # last block and its gate
            mark("final")
            if gates:
                o_lo, b_o, ib_o = gates.popleft()
                gate_final(o_lo, b_o, ib_o, last=True)
            flush_point(last=True)
            flush_point(last=True)

    nc.compile()
    return nc


def build_reps(BPC, N, D, gamma, reps=6):
    return build(BPC, N, D, gamma, reps=reps)


_CACHE = {}


def _get_nc(BPC, N, D, gamma):
    key = (BPC, N, D, float(gamma))
    if key not in _CACHE:
        _CACHE[key] = build(BPC, N, D, float(gamma))
    return _CACHE[key]


def make_in_maps(x1, x2, Wq, bq, Wk, bk, Wv, bv, W1, b1, W2, b2, W3, b3,
                 n_cores=N_CORES):
    """Host-side prep: shard over batch, transpose, cast fp8, fold weights."""
    f8 = ml_dtypes.float8_e4m3
    B, N, D = x1.shape
    DC = D // P
    Wq = np.asarray(Wq, np.float32)
    Wk = np.asarray(Wk, np.float32)
    M = Wq.T @ Wk                      # q' = x1 @ M ; energy = q' @ x2^T
    c = np.asarray(bq, np.float32) @ Wk
    Ws = (W1 + W2 + W3).astype(np.float32)
    bsum = (b1 + b2 + b3).astype(np.float32)

    def r_bias(v):  # [D] -> [128, DC] with v[c*128+p] at [p, c]
        return np.ascontiguousarray(v.reshape(DC, P).T).astype(np.float32)

    bvb = np.broadcast_to((WSC * np.asarray(bv, np.float32)).reshape(1, 2, 512),
                          (P, 2, 512))
    shared = {
        "m_t": np.ascontiguousarray(WSC_M * M).astype(f8),
        "wv_t": np.ascontiguousarray(WSC * Wv.T).astype(f8),
        "ws_t": np.ascontiguousarray(WSC * Ws.T).astype(f8),
        "b_all": np.ascontiguousarray(np.stack(
            [r_bias(WSC_Q * c), r_bias(0.0 * np.asarray(bv, np.float32)),
             r_bias(0.5 * bsum)], axis=1)),
        "bv_bc": np.ascontiguousarray(bvb).astype(ml_dtypes.bfloat16),
    }
    bpc = B // n_cores
    in_maps = []
    for cc in range(n_cores):
        sl = slice(cc * bpc, (cc + 1) * bpc)
        in_maps.append({
            "x1t_8": np.ascontiguousarray(x1[sl].transpose(0, 2, 1)).astype(f8),
            "x2t_8": np.ascontiguousarray(x2[sl].transpose(0, 2, 1)).astype(f8),
            **shared,
        })
    return in_maps


def kernel(x1, x2, Wq, bq, Wk, bk, Wv, bv, W1, b1, W2, b2, W3, b3, gamma):
    from concourse.bass_utils import run_bass_kernel_spmd

    x1 = np.asarray(x1, dtype=np.float32)
    x2 = np.asarray(x2, dtype=np.float32)
    B, N, D = x1.shape
    bpc = B // N_CORES
    nc = _get_nc(bpc, N, D, float(np.asarray(gamma).reshape(-1)[0]))
    in_maps = make_in_maps(x1, x2, np.asarray(Wq), np.asarray(bq),
                           np.asarray(Wk), np.asarray(bk),
                           np.asarray(Wv), np.asarray(bv),
                           np.asarray(W1), np.asarray(b1),
                           np.asarray(W2), np.asarray(b2),
                           np.asarray(W3), np.asarray(b3))
    out = np.empty((B, N, D), np.float32)
    # transient axon/NRT glitches occasionally corrupt a run (non-finite
    # values); the kernel itself is deterministic, so retry on detection
    for attempt in range(3):
        res = run_bass_kernel_spmd(nc, in_maps, list(range(N_CORES)))
        for c in range(N_CORES):
            out[c * bpc:(c + 1) * bpc] = \
                res.results[c]["out"].astype(np.float32).transpose(0, 2, 1)
        if np.isfinite(out).all():
            break
    # device computed fin = (tanh((s+bs)/2) + 1) * out; the gamma/2 factor
    # and the residual are applied here
    g2 = 0.5 * float(np.asarray(gamma).reshape(-1)[0])
    out = out * g2 + x1
    return out


# revision 69
# speedup vs baseline: 1.0015x; 1.0015x over previous
"""Trainium2 Bass kernel for batched cross-attention + multiscale sigmoid gate.

Reference computation (per batch b):
    q = x1 @ Wq.T + bq ; k = x2 @ Wk.T + bk ; v = x2 @ Wv.T + bv
    attn = softmax(q @ k.T, axis=-1)              (unscaled)
    out = attn @ v
    s = out @ (W1+W2+W3).T + (b1+b2+b3)
    out = out * sigmoid(s)
    return gamma * out + x1

Key algebraic fold: softmax is invariant to per-row constants, so
    softmax(q k^T) = softmax(x1 (Wq^T Wk) x2^T + 1 (bq^T Wk) x2^T)
(the x1 Wq^T bk and bq.bk terms are constant along the softmax axis and
cancel). The kernel computes q' = x1 @ M + c with M = Wq^T Wk and
c = bq @ Wk folded on the host, and uses the already-resident fp8 x2
directly as the key matrix: the entire K projection (128 of 1024 matmul
instructions per batch) and its drains vanish. bv is folded into the V
drain as a broadcast-tensor add, so the PV drains are bias-free wide ops.

Strategy: pure data-parallel over batch (16 batches -> 8 cores x 2), no
collectives. Everything on-chip is kept transposed ([feature, token]) so
all matmuls contract over the partition dim with zero on-device
transposes. ALL matmuls run fp8e4m3 DoubleRow (2x contraction tiles per
instruction): Q' projection, QK^T energy (keys = raw x2), PV, and the
gate. Weights are pre-scaled x32 on the host so fp8 encodings stay out
of the subnormal range; every descale folds into an existing epilogue.

Pipeline: two-deep deferral -- in each attention window (energy of block
ib) the PV of block ib-1 and the gate of block ib-2 are flushed, so PV
drains get a full extra window before the gate contracts over them and
nothing on the slow queues is latency-critical. At a batch boundary the
previous batch's last PV+gate run inside the first attention window
(v_sb is double-buffered for the WAR hazard); the first batch instead
defers its pb3 V-projection groups there.

Engine assignment (GPSIMD/Pool cannot touch PSUM and cannot run
tensor_scalar/STT on TRN2 -- hardware-verified constraints):
  ScalarE/ACT: exps (high-priority: latency-critical), Q' drains, all
    four wide PV drains, tanh (sigmoid via (tanh+1)/2, same ACT table as
    Exp so no table reloads).
  VectorE/DVE: den accumulation (1024-wide bf16 2x adds), reciprocal,
    softmax normalize (5 of 8 j-pair TTs), V drains (TT + broadcast bv),
    wide fin STTs.
  GpSimd/Pool: partition all-reduce of the denominator and the last 3
    normalize pairs.
Output is written fp8 (the gated path is ~3% of the result); the
gamma/2 factor and the residual add (+x1) are applied on the host.

Numerics (hardware-validated): full-output rel err 3.7e-3 vs f32
reference (budget 2e-2) -- slightly better than a separate-Q/K baseline
since only one fp8 weight quantization enters the energy path.
TimelineSim per-core estimate: 226.4us (baseline 290.8us).
"""

import math

import numpy as np
import ml_dtypes

import concourse.tile as tile
from concourse import mybir, bacc
from concourse.bass_isa import ReduceOp

P = 128
F32 = mybir.dt.float32
BF16 = mybir.dt.bfloat16
F8 = mybir.dt.float8e4
AF = mybir.ActivationFunctionType
OP = mybir.AluOpType
DR = mybir.MatmulPerfMode.DoubleRow

# full problem shape (hardcoded per harness contract)
B_FULL, N_FULL, D_FULL = 16, 2048, 1024
MARKS = None  # debug: list to receive (label, next_instr_id) emission marks
N_CORES = 8
SHIFT = 64.0
WSC = 32.0            # host-side weight scale (fp8 subnormal dodge)
WSC_M = 32.0          # scale for M = Wq^T Wk
WSC_Q = 32.0          # scale of q' as stored in fp8


def build(BPC, N, D, gamma, shift=SHIFT, reps=1):
    """Build the per-core Bass graph. BPC = batches per core."""
    DC = D // P          # feature chunks of 128
    NJ = N // P          # key tiles of 128
    PB = min(512, N)     # projection n-block
    NPB = N // PB
    IB = min(512, N)     # attention i-block (query block)
    NIB = N // IB
    KH = math.ceil(D / 512)  # V-projection k halves
    assert DC % 2 == 0 and NJ % 2 == 0

    nc = bacc.Bacc("TRN2", target_bir_lowering=False, debug=False,
                   num_devices=N_CORES)

    x1t_d = nc.declare_dram_parameter("x1t_8", [BPC, D, N], F8, isOutput=False)
    x2t_d = nc.declare_dram_parameter("x2t_8", [BPC, D, N], F8, isOutput=False)
    m_d = nc.declare_dram_parameter("m_t", [D, D], F8, isOutput=False)
    wv_d = nc.declare_dram_parameter("wv_t", [D, D], F8, isOutput=False)
    ws_d = nc.declare_dram_parameter("ws_t", [D, D], F8, isOutput=False)
    # bias slots: 0 = WSC_Q * (bq @ Wk), 1 = unused, 2 = (b1+b2+b3)/2
    ball_d = nc.declare_dram_parameter("b_all", [P, 3, DC], F32, isOutput=False)
    # WSC*bv broadcast across partitions, [P, KH, 512] (feature = kh*512+m):
    # added at the V drain so the PV drains are bias-free wide ops
    bvb_d = nc.declare_dram_parameter("bv_bc", [P, KH, 512], BF16,
                                      isOutput=False)
    out_ext = nc.declare_dram_parameter("out", [BPC, D, N], F8, isOutput=True)

    def r3(ap):  # [D, N] dram view -> [p, dc, n]
        return ap.rearrange("(c p) n -> p c n", p=P)

    def mark(label):
        if MARKS is not None:
            MARKS.append((label, nc.next_id()))

    def mm8(pst, lhsT3, rhs3, start, stop):
        """fp8 DoubleRow matmul over 2 contraction chunk-tiles."""
        nc.tensor.matmul(pst, lhsT=lhsT3, rhs=rhs3, start=start, stop=stop,
                         perf_mode=DR)

    with tile.TileContext(nc) as tc:
        with (
            tc.tile_pool(name="w8", bufs=1) as w8,
            tc.tile_pool(name="consts", bufs=1) as consts,
            tc.tile_pool(name="xin", bufs=4) as xin,
            tc.tile_pool(name="x2r", bufs=2) as x2r,
            tc.tile_pool(name="kv", bufs=1) as kvpool,
            tc.tile_pool(name="vp", bufs=2) as vpool,
            tc.tile_pool(name="pall", bufs=2) as pall_pool,
            tc.tile_pool(name="p8", bufs=2) as p8_pool,
            tc.tile_pool(name="obf", bufs=2) as obf_pool,
            tc.tile_pool(name="small", bufs=2) as small,
            tc.tile_pool(name="gp", bufs=5) as gpool,
            tc.tile_pool(name="fin", bufs=4) as finpool,
            # all PSUM flows through 2-bank [P, 2, IB] tiles: 4 bufs span
            # the whole 16KB, doubling the rotation window vs 1-bank tiles
            tc.tile_pool(name="ps2", bufs=4, space="PSUM") as ps2p,
        ):
            # constants / biases
            negshift = consts.tile([P, 1], F32)
            nc.vector.memset(negshift[:], -shift)

            # weights: fp8, resident for the whole kernel (batch-invariant).
            # Startup: column-sliced DMAs ordered so the first Q' matmul
            # group's operands (M columns 0:256 + x1 chunk half) land first;
            # wv/ws stream in behind the first projection groups.
            m_sb = w8.tile([P, DC, D], F8, tag="m")
            wv_sb = w8.tile([P, DC, D], F8, tag="wv")
            ws_sb = w8.tile([P, DC, D], F8, tag="ws")
            H = DC // 2
            nsl0 = slice(0, PB)
            x1t0 = xin.tile([P, DC, PB], F8, tag="xin")
            x2_sb0 = x2r.tile([P, DC, N], F8, tag="x2")
            # startup critical path: the first Q' mm8 needs only x1 chunks
            # 0:2 + M columns 0:128. Spread the first DMAs across the
            # SP/ACT/DVE dispatch queues (each dispatch is ~0.6us serial per
            # queue) and order by first use.
            m3 = r3(m_d.ap())
            x13 = r3(x1t_d[0])
            b_all = consts.tile([P, 3, DC], F32)
            # HWDGE descriptor generation is the serial resource here
            # (~0.6us per DMA across both queues): few, large transfers,
            # ordered by first use. b_all is tiny and gates the first Q
            # drain, so it leads the ACT queue.
            nc.sync.dma_start(x1t0[:, 0:H], x13[:, 0:H, nsl0])
            # first M column block via SWDGE (gpsimd): its generation runs
            # on the idle Pool engine, off the serial HWDGE path, so the
            # very first matmul's weights land ~1.5us earlier
            nc.gpsimd.dma_start(m_sb[:, :, 0:P], m3[:, :, 0:P])
            nc.scalar.dma_start(m_sb[:, :, P:2 * P], m3[:, :, P:2 * P])
            nc.scalar.dma_start(b_all[:], ball_d[:])
            nc.sync.dma_start(x1t0[:, H:], x13[:, H:, nsl0])
            nc.scalar.dma_start(m_sb[:, :, 2 * P:4 * P], m3[:, :, 2 * P:4 * P])
            nc.sync.dma_start(m_sb[:, :, 4 * P:D], m3[:, :, 4 * P:D])
            bv_bc = consts.tile([P, KH, 512], BF16)
            nc.scalar.dma_start(bv_bc[:], bvb_d[:])
            nc.sync.dma_start(x2_sb0[:, :, nsl0], r3(x2t_d[0])[:, :, nsl0])

            def emit_pv(p_lo, v_cur, last=False):
                # out = (P @ V')/32 + bv; kc pairs share a 2-bank PSUM
                # tile. Drains go to GpSimd: with the gate deferred a full
                # window, nothing on the Pool queue is latency-critical.
                # For the tail flush (last=True) the drains rotate over
                # ACT/DVE/Pool since every engine is idle by then.
                out_lo = obf_pool.tile([P, DC, IB], F8, tag="obf")
                mark("flushPV")
                # kc2 descending: the first groups' drains go to ACT (idle
                # right after the exps), the last to DVE; the gate also
                # contracts descending, so every chunk pair is drained just
                # ahead of its first use
                for kc2 in range(DC // 2 - 1, -1, -1):
                    o_ps = ps2p.tile([P, 2, IB], F32, tag="ps2")
                    for h in (0, 1):
                        kc = 2 * kc2 + h
                        kh, ko = divmod(kc * P, 512)
                        for jp in range(NJ // 2):
                            mm8(o_ps[:, h],
                                v_cur[:, 2 * jp:2 * jp + 2, kh, ko:ko + P],
                                p_lo[:, 2 * jp:2 * jp + 2, :],
                                start=(jp == 0), stop=(jp == NJ // 2 - 1))
                    # bias-free wide drain (bv rides in v_sb): the last
                    # chunk pair goes to ACT so its PSUM tile -- the next
                    # gate group's buffer -- frees promptly; GPSIMD cannot
                    # read PSUM on TRN2.
                    osl = out_lo[:, 2 * kc2:2 * kc2 + 2, :]
                    if not last:
                        nc.scalar.activation(osl, o_ps[:], AF.Identity,
                                             scale=1.0 / WSC)
                    else:
                        # (all drains DVE in the tail: ACT must be free to
                        # start the final tanh chain immediately)
                        nc.vector.tensor_scalar(osl, o_ps[:], 1.0 / WSC,
                                                None, OP.mult)
                return out_lo

            def gate_final(out_lo, b_o, ib, last=False):
                isl = slice(ib * IB, (ib + 1) * IB)
                o3 = r3(out_ext[b_o])
                mark("flushGate")
                for ec2 in range(DC // 2):
                    g_ps = ps2p.tile([P, 2, IB], F32, tag="ps2")
                    for h in (0, 1):
                        ec = 2 * ec2 + h
                        for dc2 in range(DC // 2 - 1, -1, -1):
                            mm8(g_ps[:, h],
                                ws_sb[:, 2 * dc2:2 * dc2 + 2, ec * P:(ec + 1) * P],
                                out_lo[:, 2 * dc2:2 * dc2 + 2, :],
                                start=(dc2 == DC // 2 - 1), stop=(dc2 == 0))
                    fin = finpool.tile([P, 2, IB], F8, tag="fin")
                    g_sb = gpool.tile([P, 2, IB], BF16, tag="g")
                    for h in (0, 1):
                        ec = 2 * ec2 + h
                        # sigmoid(x) = 0.5*tanh(x/2) + 0.5 ; Tanh shares the
                        # ACT table with Exp/Identity, so no table reloads.
                        # bias slot 2 is bs/2; fin = (tanh+1)*out and the
                        # remaining gamma/2 factor applies on the host.
                        nc.scalar.activation(g_sb[:, h], g_ps[:, h], AF.Tanh,
                                             bias=b_all[:, 2, ec:ec + 1],
                                             scale=1.0 / (2.0 * WSC))
                        if last:
                            # tail: per-half fin + DMA pipeline with the
                            # tanh chain instead of waiting for the pair
                            nc.vector.scalar_tensor_tensor(
                                fin[:, h], g_sb[:, h], 1.0,
                                out_lo[:, ec, :], OP.add, OP.mult)
                            nc.sync.dma_start(o3[:, ec, isl], fin[:, h])
                    if not last:
                        # one wide fin per pair on DVE (GPSIMD cannot STT);
                        # deprioritized: it only feeds the output DMA
                        with tc.high_priority(offset=-300):
                            nc.vector.scalar_tensor_tensor(
                                fin[:], g_sb[:], 1.0,
                                out_lo[:, 2 * ec2:2 * ec2 + 2, :],
                                OP.add, OP.mult)
                        # one 2-chunk DMA per pair
                        nc.sync.dma_start(o3[:, 2 * ec2:2 * ec2 + 2, isl],
                                          fin[:])

            # Two-deep software pipeline: at any flush point the pending
            # PV (one window behind) is emitted, then ONE queued gate
            # (two windows behind, FIFO). PV drains thus get a whole
            # extra window before the gate contracts over them. At a
            # batch boundary the PV flush and its gate split across two
            # windows so the first attention window keeps PE fed.
            import collections as _c
            first = True
            pv_pending = None    # (p_lo, v_sb, b, ib)
            gates = _c.deque()   # queued (out_lo, b, ib)

            def flush_point(last=False, gates_too=True):
                nonlocal pv_pending
                if pv_pending is not None:
                    p_lo, v_cur, b_o, ib_o = pv_pending
                    gates.append((emit_pv(p_lo, v_cur, last=last), b_o, ib_o))
                    pv_pending = None
                if gates_too and gates:
                    o_lo, b_o, ib_o = gates.popleft()
                    gate_final(o_lo, b_o, ib_o, last=last)

            pre_x = None     # next batch's first x1 tile, prefetched
            blist = [bb for _ in range(reps) for bb in range(BPC)]
            for bi, b in enumerate(blist):
                last_batch = bi == len(blist) - 1
                # ---- phase 1: Q' + V projections (all fp8 DoubleRow) ----
                qt_sb = kvpool.tile([P, DC, N], F8, tag="qt")
                if first:
                    x2_sb = x2_sb0
                else:
                    x2_sb = pre_x2
                v_sb = None  # allocated in pb0 (pool is double-buffered)
                x1_tiles = {}
                defer_v3 = bi == 0  # first batch: pb3 V runs in window ib0

                def prefetch_rest():
                    # dispatch the whole batch's remaining x tiles up
                    # front, alternating HWDGE queues: transfers overlap
                    # the full 27us projection phase
                    # all on the SP queue: scalar.dma_start dispatches
                    # occupy the ACT sequencer (~0.7us each) and delay the
                    # Q drains behind them
                    for pp in range(1, NPB):
                        ns = slice(pp * PB, (pp + 1) * PB)
                        x1p = xin.tile([P, DC, PB], F8, tag="xin")
                        nc.sync.dma_start(x1p[:], r3(x1t_d[b])[:, :, ns])
                        nc.sync.dma_start(x2_sb[:, :, ns],
                                          r3(x2t_d[b])[:, :, ns])
                        x1_tiles[pp] = x1p

                for pb in range(NPB):
                    nsl = slice(pb * PB, (pb + 1) * PB)
                    mark(f"b{bi}.pb{pb}")
                    if pb == 0:
                        x1t = x1t0 if first else pre_x
                        pre_x = None
                    else:
                        x1t = x1_tiles.pop(pb)

                    def q_pair(ec2):
                        # two Q' chunks in one 2-bank tile, ACT drains
                        pst = ps2p.tile([P, 2, PB], F32, tag="ps2")
                        for h in (0, 1):
                            e = 2 * ec2 + h
                            for dc2 in range(DC // 2):
                                mm8(pst[:, h],
                                    m_sb[:, 2 * dc2:2 * dc2 + 2, e * P:(e + 1) * P],
                                    x1t[:, 2 * dc2:2 * dc2 + 2, :],
                                    start=(dc2 == 0), stop=(dc2 == DC // 2 - 1))
                        for h in (0, 1):
                            e = 2 * ec2 + h
                            nc.scalar.activation(qt_sb[:, e, nsl],
                                                 pst[:, h], AF.Identity,
                                                 bias=b_all[:, 0, e:e + 1],
                                                 scale=WSC_Q / WSC_M)

                    def v_pair(js, vpb=None, pool_drain=False):
                        # V chunk [j, :] for one j-tile: both 512-wide k
                        # halves in one 2-bank tile, drained as a single
                        # 1024-wide copy (no bias; bv added at the PV
                        # drain). Deferred pairs drain on Pool to keep the
                        # attention-window DVE (softmax chain) clean.
                        if vpb is None:
                            vpb = pb
                        jt = vpb * (PB // P) + js
                        tok = slice(vpb * PB + js * P, vpb * PB + (js + 1) * P)
                        pst = ps2p.tile([P, 2, PB], F32, tag="ps2")
                        for kh in range(KH):
                            k0 = kh * 512
                            for dc2 in range(DC // 2):
                                mm8(pst[:, kh],
                                    x2_sb[:, 2 * dc2:2 * dc2 + 2, tok],
                                    wv_sb[:, 2 * dc2:2 * dc2 + 2, k0:k0 + 512],
                                    start=(dc2 == 0), stop=(dc2 == DC // 2 - 1))
                        nc.vector.tensor_tensor(v_sb[:, jt], pst[:],
                                                 bv_bc[:], OP.add)

                    if first and pb == 0:
                        # staged order: Q' groups first (only M + x1 gate
                        # the start), weight DMAs between groups
                        q_pair(0)
                        wv3 = r3(wv_d.ap())
                        nc.sync.dma_start(wv_sb[:, :, 0:512],
                                          wv3[:, :, 0:512])
                        nc.sync.dma_start(wv_sb[:, :, 512:D],
                                          wv3[:, :, 512:D])
                        prefetch_rest()
                        q_pair(1)
                        q_pair(2)
                        q_pair(3)
                        v_sb = vpool.tile([P, NJ, KH, 512], F8, tag="v")
                        for js in range(PB // P):
                            v_pair(js)
                        nc.sync.dma_start(ws_sb[:], r3(ws_d.ap()))
                        first = False
                    elif pb == 0:
                        # no flush here: the previous batch's PV(ib3) and
                        # gate(ib2) run inside window ib0 (v_sb is
                        # double-buffered so the new batch's V drains
                        # don't wait on PV(ib3) reading the old buffer)
                        v_sb = vpool.tile([P, NJ, KH, 512], F8, tag="v")
                        q_pair(0)
                        prefetch_rest()
                        for ec2 in range(1, DC // 2):
                            q_pair(ec2)
                            v_pair(ec2 - 1)
                        v_pair(DC // 2 - 1)
                    elif defer_v3 and pb == NPB - 1:
                        # first batch has no flush work in its first
                        # attention window; its PE would run ahead of the
                        # softmax chain and throttle on norm production.
                        # Keep pb3's V groups back and emit them there.
                        for ec2 in range(DC // 2):
                            q_pair(ec2)
                        pend_v = v_pair
                    else:
                        # interleave Q' pairs with V pairs so drains
                        # rotate over ACT and DVE
                        for ec2 in range(DC // 2):
                            q_pair(ec2)
                            v_pair(ec2)

                # ---- phase 2: attention + gate, per i-block ----
                for ib in range(NIB):
                    isl = slice(ib * IB, (ib + 1) * IB)
                    if ib == NIB - 1 and not last_batch:
                        # prefetch the next batch's first x1 tile and x2
                        # slice so its projections start without a DMA wait
                        nb = blist[bi + 1]
                        x1p = xin.tile([P, DC, PB], F8, tag="xin")
                        nc.sync.dma_start(x1p[:], r3(x1t_d[nb])[:, :, 0:PB])
                        pre_x2 = x2r.tile([P, DC, N], F8, tag="x2")
                        nc.sync.dma_start(pre_x2[:, :, 0:PB],
                                          r3(x2t_d[nb])[:, :, 0:PB])
                        pre_x = x1p
                    mark(f"b{bi}.ib{ib}.energy")
                    p_all = pall_pool.tile([P, NJ, IB], BF16, tag="pall")
                    den2 = small.tile([P, 2, IB], BF16, tag="den2")

                    # pass A: energy (fp8 DR, keys = raw x2) into 2-bank
                    # PSUM tiles so a single exp covers two j-tiles;
                    # denominator accumulated as 1024-wide bf16 2x-mode
                    # adds on DVE into den2, folded once at the end
                    for jh in range(NJ // 2):
                        ps2 = ps2p.tile([P, 2, IB], F32, tag="ps2")
                        for h in (0, 1):
                            j = 2 * jh + h
                            for dc2 in range(DC // 2):
                                mm8(ps2[:, h],
                                    x2_sb[:, 2 * dc2:2 * dc2 + 2,
                                          j * P:(j + 1) * P],
                                    qt_sb[:, 2 * dc2:2 * dc2 + 2, isl],
                                    start=(dc2 == 0),
                                    stop=(dc2 == DC // 2 - 1))
                        with tc.high_priority(offset=400):
                            nc.scalar.activation(
                                p_all[:, 2 * jh:2 * jh + 2, :],
                                ps2[:], AF.Exp,
                                bias=negshift[:, 0:1],
                                scale=1.0 / WSC_Q)
                        if jh == 0:
                            nc.vector.tensor_copy(den2[:],
                                                  p_all[:, 0:2, :])
                        else:
                            nc.vector.tensor_tensor(
                                den2[:], den2[:],
                                p_all[:, 2 * jh:2 * jh + 2, :], OP.add)

                    den_a = small.tile([P, IB], BF16, tag="dena")
                    nc.vector.tensor_tensor(den_a[:], den2[:, 0], den2[:, 1],
                                            OP.add)
                    # partition all-reduce on GpSimd (sum + broadcast in one
                    # op), then reciprocal on DVE.
                    den_all = small.tile([P, IB], F32, tag="denbf")
                    nc.gpsimd.partition_all_reduce(den_all[:], den_a[:], P,
                                                   ReduceOp.add)
                    rec_bc = small.tile([P, IB], F32, tag="recbc")
                    nc.vector.reciprocal(rec_bc[:], den_all[:])

                    # normalize P and cast to fp8, entirely on DVE and
                    # emitted BEFORE the flush: it only depends on recip
                    # (which lands early) and DVE carries no flush ops, so
                    # p_lo completes a full window before PV consumes it.
                    # normalize P pairs: the Pool share (tail pairs) goes
                    # ahead of the flush (it waits only on recip; fins have
                    # slack), the DVE share after it (so the flush's PV
                    # drains run early and free their PSUM tiles for the
                    # next window's rotation).
                    mark(f"b{bi}.ib{ib}.norm")
                    p_lo = p8_pool.tile([P, NJ, IB], F8, tag="p8")
                    rec2 = rec_bc[:].unsqueeze(1).broadcast_to((P, 2, IB))
                    n_gp = 3 if NJ >= 16 else 0
                    for jh in range(NJ // 2 - n_gp, NJ // 2):
                        nc.gpsimd.tensor_tensor(
                            p_lo[:, 2 * jh:2 * jh + 2, :],
                            p_all[:, 2 * jh:2 * jh + 2, :],
                            rec2, OP.mult)
                    flush_point()
                    with tc.high_priority(offset=400):
                        for jh in range(NJ // 2 - n_gp):
                            nc.vector.tensor_tensor(
                                p_lo[:, 2 * jh:2 * jh + 2, :],
                                p_all[:, 2 * jh:2 * jh + 2, :],
                                rec2, OP.mult)
                    if ib == 0 and defer_v3:
                        for js in range(PB // P):
                            pend_v(js, vpb=NPB - 1)
                        defer_v3 = False
                    if last_batch and ib == NIB - 1 and gates:
                        # pull the ib2 gate into this window: its tanh
                        # chain then overlaps the tail's PV matmuls
                        o_lo, b_o, ib_o = gates.popleft()
                        gate_final(o_lo, b_o, ib_o)
                    pv_pending = (p_lo, v_sb, b, ib)

            # tail: the pending gate first (its inputs are long ready, so
            # its tanh chain overlaps PV(ib3)'s matmuls), then PV of the
            # last block and its gate
            mark("final")
            if gates:
                o_lo, b_o, ib_o = gates.popleft()
                gate_final(o_lo, b_o, ib_o, last=True)
            flush_point(last=True)
            flush_point(last=True)

    nc.compile()
    return nc


def build_reps(BPC, N, D, gamma, reps=6):
    return build(BPC, N, D, gamma, reps=reps)


_CACHE = {}


def _get_nc(BPC, N, D, gamma):
    key = (BPC, N, D, float(gamma))
    if key not in _CACHE:
        _CACHE[key] = build(BPC, N, D, float(gamma))
    return _CACHE[key]


def make_in_maps(x1, x2, Wq, bq, Wk, bk, Wv, bv, W1, b1, W2, b2, W3, b3,
                 n_cores=N_CORES):
    """Host-side prep: shard over batch, transpose, cast fp8, fold weights."""
    f8 = ml_dtypes.float8_e4m3
    B, N, D = x1.shape
    DC = D // P
    Wq = np.asarray(Wq, np.float32)
    Wk = np.asarray(Wk, np.float32)
    M = Wq.T @ Wk                      # q' = x1 @ M ; energy = q' @ x2^T
    c = np.asarray(bq, np.float32) @ Wk
    Ws = (W1 + W2 + W3).astype(np.float32)
    bsum = (b1 + b2 + b3).astype(np.float32)

    def r_bias(v):  # [D] -> [128, DC] with v[c*128+p] at [p, c]
        return np.ascontiguousarray(v.reshape(DC, P).T).astype(np.float32)

    bvb = np.broadcast_to((WSC * np.asarray(bv, np.float32)).reshape(1, 2, 512),
                          (P, 2, 512))
    shared = {
        "m_t": np.ascontiguousarray(WSC_M * M).astype(f8),
        "wv_t": np.ascontiguousarray(WSC * Wv.T).astype(f8),
        "ws_t": np.ascontiguousarray(WSC * Ws.T).astype(f8),
        "b_all": np.ascontiguousarray(np.stack(
            [r_bias(WSC_Q * c), r_bias(0.0 * np.asarray(bv, np.float32)),
             r_bias(0.5 * bsum)], axis=1)),
        "bv_bc": np.ascontiguousarray(bvb).astype(ml_dtypes.bfloat16),
    }
    bpc = B // n_cores
    in_maps = []
    for cc in range(n_cores):
        sl = slice(cc * bpc, (cc + 1) * bpc)
        in_maps.append({
            "x1t_8": np.ascontiguousarray(x1[sl].transpose(0, 2, 1)).astype(f8),
            "x2t_8": np.ascontiguousarray(x2[sl].transpose(0, 2, 1)).astype(f8),
            **shared,
        })
    return in_maps


def kernel(x1, x2, Wq, bq, Wk, bk, Wv, bv, W1, b1, W2, b2, W3, b3, gamma):
    from concourse.bass_utils import run_bass_kernel_spmd

    x1 = np.asarray(x1, dtype=np.float32)
    x2 = np.asarray(x2, dtype=np.float32)
    B, N, D = x1.shape
    bpc = B // N_CORES
    nc = _get_nc(bpc, N, D, float(np.asarray(gamma).reshape(-1)[0]))
    in_maps = make_in_maps(x1, x2, np.asarray(Wq), np.asarray(bq),
                           np.asarray(Wk), np.asarray(bk),
                           np.asarray(Wv), np.asarray(bv),
                           np.asarray(W1), np.asarray(b1),
                           np.asarray(W2), np.asarray(b2),
                           np.asarray(W3), np.asarray(b3))
    out = np.empty((B, N, D), np.float32)
    # transient axon/NRT glitches occasionally corrupt a run (non-finite
    # values); the kernel itself is deterministic, so retry on detection
    for attempt in range(3):
        res = run_bass_kernel_spmd(nc, in_maps, list(range(N_CORES)))
        for c in range(N_CORES):
            out[c * bpc:(c + 1) * bpc] = \
                res.results[c]["out"].astype(np.float32).transpose(0, 2, 1)
        if np.isfinite(out).all():
            break
    # device computed fin = (tanh((s+bs)/2) + 1) * out; the gamma/2 factor
    # and the residual are applied here
    g2 = 0.5 * float(np.asarray(gamma).reshape(-1)[0])
    out = out * g2 + x1
    return out
